# revision 29
# baseline (speedup 1.0000x reference)
"""Trainium2 Bass kernel for GNN message passing:

    h = segment_sum(x[src] * (edge_basis @ W.T + b), dst, num_segments=N)

Strategy (node-sharded, sort-based; no collectives — each core owns its
output rows exclusively):
  - Host: sort edges by dst; core c owns the contiguous node range
    [c*N/8, (c+1)*N/8). Within a core, nodes are grouped into blocks of
    128; each block's (contiguous, because sorted) edge list is padded to
    tiles of 128 edges. Blocks are processed in per-core descending-size
    order so the shared (SPMD-identical) tile schedule T_list[j] =
    max_over_cores(j-th largest block) stays tight (~3% padding); the
    host un-permutes the output blocks afterwards. Per core the host
    materializes one packed stream (single DMA per block, issued on
    alternating HWDGE rings):
      * st  [128, TT*192] bf16 : per block, edge_basis tiles TRANSPOSED
        ([r, 128e], consumed as matmul weights) followed by x[src]
        gathered rows ([128e, 64], edge on partition).
      * sall [128, TT*128] f8  : HOST-PREBUILT one-hot scatter tiles
        S[e, n] = (dst_e - block_base == n), fp8e4 (exact 0/1), kept
        SBUF-RESIDENT (loaded once, outside the steady-state loop —
        topology-derived constant, reusable across passes).
      * wt  [128, 64]     bf16 : W.T (matmul rhs, resident).
    Pad edges have eb = 0, xs = 0 -> contribute exactly 0.
  - Device per tile (128 edges), software-pipelined with SKEW chunks:
      PE:  filt[128e,64d](psum) = ebT_tile.T @ WT   (FWL bf16 weight swap)
      DVE: m(bf16 sbuf) = xs * filt  -- reads PSUM f32 directly,
           chunk-batched (no ACT copy stage)
      PE:  psum_h[128n,64d] += S_tile.T @ m         (fp8 FWL weight swap,
           accumulate per block)
    Per block: ACT copies psum_h into a resident SBUF strip; every 8
    blocks one DMA stores that h slice (overlapped with compute); host
    de-interleaves to [N/8, 64].
  Accuracy: inputs quantized to bf16, one-hot exact in fp8, accumulation
  in f32 PSUM; rel RMS error ~3.5e-3 vs the f32 reference.
"""

import math
from contextlib import ExitStack

import numpy as np
import ml_dtypes

import concourse.bass as bass
import concourse.bacc as bacc
import concourse.tile as tile
from concourse import mybir
from concourse.bass_utils import run_bass_kernel_spmd

BF16 = ml_dtypes.bfloat16
F8 = ml_dtypes.float8_e4m3  # mybir.dt.float8e4

# Problem configuration (hardcoded per the task spec).
N_NODES = 50000
N_EDGES = 800000
D_IN = 64
D_RADIAL = 128
N_CORES = 8

LAST_BUILD = None  # (nc, in_maps) of the most recent build, for test harnesses

BLK = 128          # nodes per block (= one-hot width = psum partition dim)
CHUNK_MAX = 8      # tiles per DVE batch (psum_filt = 1 bank)
SKEW = 2           # software-pipeline skew, in chunks
H_DMA_BLKS = 8     # store h strip every this many finished blocks
SORT_BLOCKS = True


def _plan(dst_sorted, n_nodes, n_cores):
    """Compute per-(core, block) edge ranges and the shared tile schedule.

    Returns (T_list, e_start, e_end, npc, n_blocks, perm):
      T_list[j]  = tiles allocated for local block j (same for all cores)
      e_start/e_end[c, j] = edge index range (into the sorted edge order)
    """
    npc = n_nodes // n_cores
    assert npc * n_cores == n_nodes
    n_blocks = math.ceil(npc / BLK)
    bounds = np.empty((n_cores, n_blocks + 1), np.int64)
    for c in range(n_cores):
        for j in range(n_blocks + 1):
            bounds[c, j] = c * npc + min(j * BLK, npc)
    e_bounds = np.searchsorted(dst_sorted, bounds.ravel()).reshape(bounds.shape)
    e_start = e_bounds[:, :-1]
    e_end = e_bounds[:, 1:]
    counts = e_end - e_start
    tiles_needed = np.maximum((counts + BLK - 1) // BLK, 1)
    if SORT_BLOCKS:
        # Each core processes its blocks in descending-size order; loop
        # position j is sized by the max over cores of the j-th largest
        # block, which is much tighter than max over cores per block.
        perm = np.argsort(-tiles_needed, axis=1, kind="stable")
        sorted_tiles = np.take_along_axis(tiles_needed, perm, axis=1)
        T_list = sorted_tiles.max(axis=0)
    else:
        perm = np.tile(np.arange(n_blocks), (n_cores, 1))
        T_list = tiles_needed.max(axis=0)
    return T_list, e_start, e_end, npc, n_blocks, perm


def _prepare_core(eb_bf, srcx, order, e_start, e_end, T_list, perm,
                  core, npc, n_blocks, d_in, d_radial):
    """Build the per-core device input arrays (see module docstring)."""
    TT = int(T_list.sum())
    idx = np.full(TT * BLK, -1, np.int64)  # into sorted-edge order
    off = 0
    for j in range(n_blocks):
        blk = int(perm[core, j])
        s, e = e_start[core, blk], e_end[core, blk]
        idx[off * BLK: off * BLK + (e - s)] = order[s:e]
        off += int(T_list[j])
    pad = idx < 0
    idxc = np.where(pad, 0, idx)

    # ebT tiles: [TT, 128e, d_radial] -> [d_radial, TT, 128e]
    ebg = eb_bf[idxc]
    ebg[pad] = 0
    ebT = ebg.reshape(TT, BLK, d_radial).transpose(2, 0, 1)

    # xs tiles: [TT, 128e, d_in] -> [128e, TT, d_in]
    xsg = srcx[idxc]                  # x[src] per original edge id
    xsg[pad] = 0
    xs = xsg.reshape(TT, BLK, d_in).transpose(1, 0, 2)

    ebs = np.ascontiguousarray(ebT).reshape(BLK, TT * BLK)
    xss = np.ascontiguousarray(xs).reshape(BLK, TT * d_in)
    return ebs, xss, idx, pad


def _chunks(T_list, n_blocks):
    """Flat chunk schedule: (block j, tile base within block, chunk size)."""
    out = []
    for j in range(n_blocks):
        Tj = int(T_list[j])
        n_chunks = math.ceil(Tj / CHUNK_MAX)
        chunk = math.ceil(Tj / n_chunks)
        base = 0
        while base < Tj:
            cs = min(chunk, Tj - base)
            out.append((j, base, cs, Tj))
            base += cs
    return out


def build_program(TT, T_list, n_blocks, d_in=D_IN, d_radial=D_RADIAL,
                  n_cores=N_CORES, has_bias=False, repeat=1, loop_n=0,
                  bench_no_dma=False, bench_no_compute=False,
                  bench_no_mul=False, bench_no_scatter=False,
                  bench_no_filt=False, bench_no_drain=False, filler=0):
    """Build + compile the SPMD Bass program (identical across cores)."""
    nc = bacc.Bacc("TRN2", target_bir_lowering=False, debug=False,
                   num_devices=n_cores)
    bf = mybir.dt.bfloat16
    f32 = mybir.dt.float32
    f8 = mybir.dt.float8e4

    eb_d = nc.dram_tensor("ebs", [BLK, TT * BLK], bf, kind="ExternalInput")
    xs_d = nc.dram_tensor("xss", [BLK, TT * d_in], bf, kind="ExternalInput")
    sall_d = nc.dram_tensor("sall", [BLK, TT * BLK], f8, kind="ExternalInput")
    wt_d = nc.dram_tensor("wt", [d_radial, d_in], bf, kind="ExternalInput")
    if has_bias:
        bb_d = nc.dram_tensor("bb", [BLK, CHUNK_MAX * d_in], f32,
                              kind="ExternalInput")
    h_d = nc.dram_tensor("h", [BLK, n_blocks * d_in], f32, kind="ExternalOutput")

    T_cap = int(max(T_list))
    sched = _chunks(T_list, n_blocks)
    NCH = len(sched)
    # Per-chunk tile-offset of its block start in the global tile order.
    blk_off = np.concatenate([[0], np.cumsum(T_list)]).astype(int)

    with TileContextCompat(nc) as tc, ExitStack() as ctx:
        const = ctx.enter_context(tc.tile_pool(name="const", bufs=1))
        ebp = ctx.enter_context(tc.tile_pool(name="ebp", bufs=7))
        xsp = ctx.enter_context(tc.tile_pool(name="xsp", bufs=8))
        msb = ctx.enter_context(tc.tile_pool(name="msb", bufs=SKEW + 3))
        pfil = ctx.enter_context(
            tc.tile_pool(name="pfil", bufs=SKEW + 2, space="PSUM"))
        ph = ctx.enter_context(tc.tile_pool(name="ph", bufs=3, space="PSUM"))
        if filler:
            pdum = ctx.enter_context(
                tc.tile_pool(name="pdum", bufs=1, space="PSUM"))

        # Resident constants: loaded ONCE, outside the steady-state loop.
        wt_t = const.tile([d_radial, d_in], bf)
        nc.sync.dma_start(wt_t[:], wt_d.ap())
        sall_t = const.tile([BLK, TT * BLK], f8)
        nc.scalar.dma_start(sall_t[:], sall_d.ap())
        if has_bias:
            bb_t = const.tile([BLK, CHUNK_MAX * d_in], f32)
            nc.sync.dma_start(bb_t[:], bb_d.ap())
        h_all = const.tile([BLK, n_blocks * d_in], f32)
        if bench_no_dma:
            eb_shared = const.tile([BLK, T_cap * BLK], bf)
            nc.vector.memset(eb_shared[:], 0)
            xs_shared = const.tile([BLK, T_cap * d_in], bf)
            nc.vector.memset(xs_shared[:], 0)

        import contextlib
        loop_cm = (tc.For_i(0, loop_n, 1,
                            hint_engines=(mybir.EngineType.PE,
                                          mybir.EngineType.DVE,
                                          mybir.EngineType.Activation,
                                          mybir.EngineType.SP),
                            staggered_reset=True)
                   if loop_n else contextlib.nullcontext())
        with loop_cm:
          for _rep in range(repeat):
            eb_tiles = {}    # block j -> eb tile (freed after last filt)
            xs_tiles = {}    # block j -> xs tile (freed after last mul)
            pf_of = {}       # chunk q -> psum filt tile
            ph_of = {}       # block j -> psum h tile
            done_blocks = 0  # blocks drained so far (for h out-DMA)

            def emit_filt(q):
                j, base, cs, Tj = sched[q]
                if j not in eb_tiles:
                    if bench_no_dma:
                        eb_tiles[j] = eb_shared
                        xs_tiles[j] = xs_shared
                    else:
                        eb_t = ebp.tile([BLK, T_cap * BLK], bf, tag="eb")
                        nc.sync.dma_start(
                            eb_t[:, :Tj * BLK],
                            eb_d.ap()[:, blk_off[j] * BLK:
                                      (blk_off[j] + Tj) * BLK])
                        xs_t = xsp.tile([BLK, T_cap * d_in], bf, tag="xs")
                        nc.scalar.dma_start(
                            xs_t[:, :Tj * d_in],
                            xs_d.ap()[:, blk_off[j] * d_in:
                                      (blk_off[j] + Tj) * d_in])
                        eb_tiles[j] = eb_t
                        xs_tiles[j] = xs_t
                if bench_no_compute or bench_no_filt:
                    return
                eb_t = eb_tiles[j]
                pf = pfil.tile([BLK, CHUNK_MAX * d_in], f32, tag="pf")
                for k in range(cs):
                    t = base + k
                    nc.tensor.matmul(pf[:, k * d_in:(k + 1) * d_in],
                                     eb_t[:, t * BLK:(t + 1) * BLK],
                                     wt_t[:], start=True, stop=True)
                pf_of[q] = pf

            def emit_scatter(q):
                nonlocal done_blocks
                j, base, cs, Tj = sched[q]
                if bench_no_compute or bench_no_scatter:
                    if (not bench_no_compute) and (q in pf_of):
                        pf_of.pop(q)
                    return
                xs_t = xs_tiles[j]
                pf = pf_of.pop(q) if not bench_no_filt else None
                if bench_no_mul or bench_no_filt:
                    m_sb = xs_t[:, base * d_in:(base + cs) * d_in]
                else:
                    m_sb = msb.tile([BLK, CHUNK_MAX * d_in], bf, tag="m")
                    if has_bias:
                        nc.vector.tensor_add(pf[:, :cs * d_in],
                                             pf[:, :cs * d_in],
                                             bb_t[:, :cs * d_in])
                    nc.vector.tensor_mul(
                        m_sb[:, :cs * d_in],
                        xs_t[:, base * d_in:(base + cs) * d_in],
                        pf[:, :cs * d_in])
                if j not in ph_of:
                    ph_of[j] = ph.tile([BLK, d_in], f32, tag="ph",
                                       name=f"psum_h_{j}")
                psum_h = ph_of[j]
                s_col0 = blk_off[j] * BLK
                for k in range(cs):
                    t = base + k
                    nc.tensor.matmul(
                        psum_h[:],
                        sall_t[:, s_col0 + t * BLK: s_col0 + (t + 1) * BLK],
                        m_sb[:, k * d_in:(k + 1) * d_in],
                        start=(t == 0), stop=(t == Tj - 1))
                if base + cs == Tj:  # block finished -> drain to SBUF strip
                    if bench_no_drain:
                        ph_of.pop(j)
                        return
                    nc.vector.tensor_copy(h_all[:, j * d_in:(j + 1) * d_in],
                                          psum_h[:])
                    ph_of.pop(j)
                    done_blocks += 1
                    if done_blocks == n_blocks:
                        nc.scalar.dma_start(h_d.ap(), h_all[:])

            if filler:
                # HAM keep-warm: dependency-free matmuls the in-order PE can
                # run during DMA-wait gaps, so the clock gate stays at 8/8.
                pdum_t = pdum.tile([BLK, d_in], f32)

                def emit_filler():
                    for _ in range(filler):
                        nc.tensor.matmul(pdum_t[:], sall_t[:, :BLK],
                                         wt_t[:, :d_in], start=True,
                                         stop=True, skip_group_check=True)

            for q in range(NCH + SKEW):
                if q < NCH:
                    emit_filt(q)
                if q >= SKEW:
                    emit_scatter(q - SKEW)
                if filler:
                    emit_filler()

    nc.compile()
    return nc


# TileContext wrapper: single place to tweak kwargs if needed.
def TileContextCompat(nc):
    return tile.TileContext(nc)


def _kernel_impl(x, edge_basis, src, dst, W, b,
                 n_nodes, d_in, d_radial, n_cores, run_fn=None):
    dst = np.asarray(dst)
    order = np.argsort(dst, kind="stable")
    dst_sorted = dst[order]
    T_list, e_start, e_end, npc, n_blocks, perm = _plan(dst_sorted, n_nodes,
                                                        n_cores)
    TT = int(T_list.sum())

    eb_bf = np.asarray(edge_basis).astype(BF16)
    srcx = np.asarray(x)[np.asarray(src)].astype(BF16)  # x gathered per edge

    has_bias = bool(np.any(np.asarray(b) != 0))

    in_maps = []
    for c in range(n_cores):
        ebs, xss, idx, pad = _prepare_core(
            eb_bf, srcx, order, e_start, e_end, T_list, perm, c, npc,
            n_blocks, d_in, d_radial)
        # rel per slot: node index within the 128-node block; pads -> 0
        # (their m is exactly 0, so the scatter target is irrelevant).
        rel_slot = np.zeros(TT * BLK, np.int64)
        valid = ~pad
        rel_slot[valid] = (dst[idx[valid]] - c * npc) % BLK
        one_hot = np.zeros((TT * BLK, BLK), np.uint8)
        one_hot[np.arange(TT * BLK), rel_slot] = 1
        sall = np.ascontiguousarray(
            one_hot.reshape(TT, BLK, BLK).transpose(1, 0, 2)
            .reshape(BLK, TT * BLK)).astype(F8)
        m = {
            "ebs": ebs,
            "xss": xss,
            "sall": sall,
            "wt": np.ascontiguousarray(np.asarray(W).T).astype(BF16),
        }
        if has_bias:
            m["bb"] = np.tile(np.asarray(b).astype(np.float32),
                              (BLK, CHUNK_MAX))
        in_maps.append(m)

    nc = build_program(TT, T_list, n_blocks, d_in, d_radial, n_cores,
                       has_bias)
    global LAST_BUILD
    LAST_BUILD = (nc, in_maps)
    if run_fn is None:
        res = run_bass_kernel_spmd(nc, in_maps, core_ids=list(range(n_cores)))
        results = res.results
    else:
        results = run_fn(nc, in_maps)

    h = np.empty((n_nodes, d_in), np.float32)
    for c in range(n_cores):
        hc = results[c]["h"].reshape(BLK, n_blocks, d_in).transpose(1, 0, 2)
        blocks = np.empty_like(hc)          # un-permute loop order -> blocks
        blocks[perm[c]] = hc
        blocks = blocks.reshape(n_blocks * BLK, d_in)
        h[c * npc:(c + 1) * npc] = blocks[:npc]
    return h


def kernel(x, edge_basis, src, dst, W, b):
    assert x.shape == (N_NODES, D_IN)
    assert edge_basis.shape == (N_EDGES, D_RADIAL)
    h = _kernel_impl(x, edge_basis, src, dst, W, b,
                     N_NODES, D_IN, D_RADIAL, N_CORES)
    return h.astype(x.dtype)


# revision 30
# speedup vs baseline: 1.0921x; 1.0921x over previous
"""Trainium2 Bass kernel for GNN message passing:

    h = segment_sum(x[src] * (edge_basis @ W.T + b), dst, num_segments=N)

Strategy (node-sharded, sort-based; no collectives — each core owns its
output rows exclusively):
  - Host: sort edges by dst; core c owns the contiguous node range
    [c*N/8, (c+1)*N/8). Within a core, nodes are grouped into blocks of
    128; each block's (contiguous, because sorted) edge list is padded to
    tiles of 128 edges. Blocks are processed in per-core descending-size
    order so the shared (SPMD-identical) tile schedule T_list[j] =
    max_over_cores(j-th largest block) stays tight (~3% padding); the
    host un-permutes the output blocks afterwards. Per core the host
    materializes one packed stream (single DMA per block, issued on
    alternating HWDGE rings):
      * st  [128, TT*192] bf16 : per block, edge_basis tiles TRANSPOSED
        ([r, 128e], consumed as matmul weights) followed by x[src]
        gathered rows ([128e, 64], edge on partition).
      * sall [128, TT*128] f8  : HOST-PREBUILT one-hot scatter tiles
        S[e, n] = (dst_e - block_base == n), fp8e4 (exact 0/1), kept
        SBUF-RESIDENT (loaded once, outside the steady-state loop —
        topology-derived constant, reusable across passes).
      * wt  [128, 64]     bf16 : W.T (matmul rhs, resident).
    Pad edges have eb = 0, xs = 0 -> contribute exactly 0.
  - Device per tile (128 edges), software-pipelined with SKEW chunks:
      PE:  filt[128e,64d](psum) = ebT_tile.T @ WT   (FWL bf16 weight swap)
      DVE: m(bf16 sbuf) = xs * filt  -- reads PSUM f32 directly,
           chunk-batched (no ACT copy stage)
      PE:  psum_h[128n,64d] += S_tile.T @ m         (fp8 FWL weight swap,
           accumulate per block)
    Per block: ACT copies psum_h into a resident SBUF strip; every 8
    blocks one DMA stores that h slice (overlapped with compute); host
    de-interleaves to [N/8, 64].
  Accuracy: inputs quantized to bf16, one-hot exact in fp8, accumulation
  in f32 PSUM; rel RMS error ~3.5e-3 vs the f32 reference.
"""

import math
from contextlib import ExitStack

import numpy as np
import ml_dtypes

import concourse.bass as bass
import concourse.bacc as bacc
import concourse.tile as tile
from concourse import mybir
from concourse.bass_utils import run_bass_kernel_spmd

BF16 = ml_dtypes.bfloat16
F8 = ml_dtypes.float8_e4m3  # mybir.dt.float8e4

# Problem configuration (hardcoded per the task spec).
N_NODES = 50000
N_EDGES = 800000
D_IN = 64
D_RADIAL = 128
N_CORES = 8

LAST_BUILD = None  # (nc, in_maps) of the most recent build, for test harnesses

BLK = 128          # nodes per block (= one-hot width = psum partition dim)
CHUNK_MAX = 8      # tiles per DVE batch (psum_filt = 1 bank)
SKEW = 2           # software-pipeline skew, in chunks
H_DMA_BLKS = 8     # store h strip every this many finished blocks
SORT_BLOCKS = True


def _plan(dst_sorted, n_nodes, n_cores):
    """Compute per-(core, block) edge ranges and the shared tile schedule.

    Returns (T_list, e_start, e_end, npc, n_blocks, perm):
      T_list[j]  = tiles allocated for local block j (same for all cores)
      e_start/e_end[c, j] = edge index range (into the sorted edge order)
    """
    npc = n_nodes // n_cores
    assert npc * n_cores == n_nodes
    n_blocks = math.ceil(npc / BLK)
    bounds = np.empty((n_cores, n_blocks + 1), np.int64)
    for c in range(n_cores):
        for j in range(n_blocks + 1):
            bounds[c, j] = c * npc + min(j * BLK, npc)
    e_bounds = np.searchsorted(dst_sorted, bounds.ravel()).reshape(bounds.shape)
    e_start = e_bounds[:, :-1]
    e_end = e_bounds[:, 1:]
    counts = e_end - e_start
    tiles_needed = np.maximum((counts + BLK - 1) // BLK, 1)
    if SORT_BLOCKS:
        # Each core processes its blocks in descending-size order; loop
        # position j is sized by the max over cores of the j-th largest
        # block, which is much tighter than max over cores per block.
        perm = np.argsort(-tiles_needed, axis=1, kind="stable")
        sorted_tiles = np.take_along_axis(tiles_needed, perm, axis=1)
        T_list = sorted_tiles.max(axis=0)
    else:
        perm = np.tile(np.arange(n_blocks), (n_cores, 1))
        T_list = tiles_needed.max(axis=0)
    return T_list, e_start, e_end, npc, n_blocks, perm


def _prepare_core(eb_bf, srcx, order, e_start, e_end, T_list, perm,
                  core, npc, n_blocks, d_in, d_radial):
    """Build the per-core device input arrays (see module docstring)."""
    TT = int(T_list.sum())
    idx = np.full(TT * BLK, -1, np.int64)  # into sorted-edge order
    off = 0
    for j in range(n_blocks):
        blk = int(perm[core, j])
        s, e = e_start[core, blk], e_end[core, blk]
        idx[off * BLK: off * BLK + (e - s)] = order[s:e]
        off += int(T_list[j])
    pad = idx < 0
    idxc = np.where(pad, 0, idx)

    # ebT tiles: [TT, 128e, d_radial] -> [d_radial, TT, 128e]
    ebg = eb_bf[idxc]
    ebg[pad] = 0
    ebT = ebg.reshape(TT, BLK, d_radial).transpose(2, 0, 1)

    # xs tiles: [TT, 128e, d_in] -> [128e, TT, d_in]
    xsg = srcx[idxc]                  # x[src] per original edge id
    xsg[pad] = 0
    xs = xsg.reshape(TT, BLK, d_in).transpose(1, 0, 2)

    ebs = np.ascontiguousarray(ebT).reshape(BLK, TT * BLK)
    xss = np.ascontiguousarray(xs).reshape(BLK, TT * d_in)
    return ebs, xss, idx, pad


def _chunks(T_list, n_blocks):
    """Flat chunk schedule: (block j, tile base within block, chunk size)."""
    out = []
    for j in range(n_blocks):
        Tj = int(T_list[j])
        n_chunks = math.ceil(Tj / CHUNK_MAX)
        chunk = math.ceil(Tj / n_chunks)
        base = 0
        while base < Tj:
            cs = min(chunk, Tj - base)
            out.append((j, base, cs, Tj))
            base += cs
    return out


def build_program(TT, T_list, n_blocks, d_in=D_IN, d_radial=D_RADIAL,
                  n_cores=N_CORES, has_bias=False, repeat=1, loop_n=0,
                  bench_no_dma=False, bench_no_compute=False,
                  bench_no_mul=False, bench_no_scatter=False,
                  bench_no_filt=False, bench_no_drain=False, filler=0):
    """Build + compile the SPMD Bass program (identical across cores)."""
    nc = bacc.Bacc("TRN2", target_bir_lowering=False, debug=False,
                   num_devices=n_cores)
    bf = mybir.dt.bfloat16
    f32 = mybir.dt.float32
    f8 = mybir.dt.float8e4

    eb_d = nc.dram_tensor("ebs", [BLK, TT * BLK], bf, kind="ExternalInput")
    xs_d = nc.dram_tensor("xss", [BLK, TT * d_in], bf, kind="ExternalInput")
    sall_d = nc.dram_tensor("sall", [BLK, TT * BLK], f8, kind="ExternalInput")
    wt_d = nc.dram_tensor("wt", [d_radial, d_in], bf, kind="ExternalInput")
    if has_bias:
        bb_d = nc.dram_tensor("bb", [BLK, CHUNK_MAX * d_in], f32,
                              kind="ExternalInput")
    h_d = nc.dram_tensor("h", [BLK, n_blocks * d_in], f32, kind="ExternalOutput")

    T_cap = int(max(T_list))
    sched = _chunks(T_list, n_blocks)
    NCH = len(sched)
    # Per-chunk tile-offset of its block start in the global tile order.
    blk_off = np.concatenate([[0], np.cumsum(T_list)]).astype(int)

    with TileContextCompat(nc) as tc, ExitStack() as ctx:
        const = ctx.enter_context(tc.tile_pool(name="const", bufs=1))
        ebp = ctx.enter_context(tc.tile_pool(name="ebp", bufs=6))
        xsp = ctx.enter_context(tc.tile_pool(name="xsp", bufs=8))
        msb = ctx.enter_context(tc.tile_pool(name="msb", bufs=SKEW + 3))
        pfil = ctx.enter_context(
            tc.tile_pool(name="pfil", bufs=SKEW + 2, space="PSUM"))
        ph = ctx.enter_context(tc.tile_pool(name="ph", bufs=3, space="PSUM"))
        if filler:
            pdum = ctx.enter_context(
                tc.tile_pool(name="pdum", bufs=1, space="PSUM"))

        # Resident constants: loaded ONCE, outside the steady-state loop.
        wt_t = const.tile([d_radial, d_in], bf)
        nc.sync.dma_start(wt_t[:], wt_d.ap())
        sall_t = const.tile([BLK, TT * BLK], f8)
        nc.scalar.dma_start(sall_t[:], sall_d.ap())
        if has_bias:
            bb_t = const.tile([BLK, CHUNK_MAX * d_in], f32)
            nc.sync.dma_start(bb_t[:], bb_d.ap())
        h_all = const.tile([BLK, n_blocks * d_in], f32)
        if bench_no_dma:
            eb_shared = const.tile([BLK, T_cap * BLK], bf)
            nc.vector.memset(eb_shared[:], 0)
            xs_shared = const.tile([BLK, T_cap * d_in], bf)
            nc.vector.memset(xs_shared[:], 0)

        import contextlib
        loop_cm = (tc.For_i(0, loop_n, 1,
                            hint_engines=(mybir.EngineType.PE,
                                          mybir.EngineType.DVE,
                                          mybir.EngineType.Activation,
                                          mybir.EngineType.SP),
                            staggered_reset=True)
                   if loop_n else contextlib.nullcontext())
        with loop_cm:
          for _rep in range(repeat):
            eb_tiles = {}    # block j -> eb tile (freed after last filt)
            xs_tiles = {}    # block j -> xs tile (freed after last mul)
            pf_of = {}       # chunk q -> psum filt tile
            ph_of = {}       # block j -> psum h tile
            done_blocks = 0  # blocks drained so far (for h out-DMA)

            def emit_filt(q):
                j, base, cs, Tj = sched[q]
                if j not in eb_tiles:
                    if bench_no_dma:
                        eb_tiles[j] = eb_shared
                        xs_tiles[j] = xs_shared
                    else:
                        eb_t = ebp.tile([BLK, T_cap * BLK], bf, tag="eb")
                        nc.sync.dma_start(
                            eb_t[:, :Tj * BLK],
                            eb_d.ap()[:, blk_off[j] * BLK:
                                      (blk_off[j] + Tj) * BLK])
                        xs_t = xsp.tile([BLK, T_cap * d_in], bf, tag="xs")
                        nc.scalar.dma_start(
                            xs_t[:, :Tj * d_in],
                            xs_d.ap()[:, blk_off[j] * d_in:
                                      (blk_off[j] + Tj) * d_in])
                        eb_tiles[j] = eb_t
                        xs_tiles[j] = xs_t
                if bench_no_compute or bench_no_filt:
                    return
                eb_t = eb_tiles[j]
                pf = pfil.tile([BLK, CHUNK_MAX * d_in], f32, tag="pf")
                for k in range(cs):
                    t = base + k
                    nc.tensor.matmul(pf[:, k * d_in:(k + 1) * d_in],
                                     eb_t[:, t * BLK:(t + 1) * BLK],
                                     wt_t[:], start=True, stop=True)
                pf_of[q] = pf

            def emit_scatter(q):
                nonlocal done_blocks
                j, base, cs, Tj = sched[q]
                if bench_no_compute or bench_no_scatter:
                    if (not bench_no_compute) and (q in pf_of):
                        pf_of.pop(q)
                    return
                xs_t = xs_tiles[j]
                pf = pf_of.pop(q) if not bench_no_filt else None
                if bench_no_mul or bench_no_filt:
                    m_sb = xs_t[:, base * d_in:(base + cs) * d_in]
                else:
                    m_sb = msb.tile([BLK, CHUNK_MAX * d_in], bf, tag="m")
                    if has_bias:
                        nc.vector.tensor_add(pf[:, :cs * d_in],
                                             pf[:, :cs * d_in],
                                             bb_t[:, :cs * d_in])
                    nc.vector.tensor_mul(
                        m_sb[:, :cs * d_in],
                        xs_t[:, base * d_in:(base + cs) * d_in],
                        pf[:, :cs * d_in])
                if j not in ph_of:
                    ph_of[j] = ph.tile([BLK, d_in], f32, tag="ph",
                                       name=f"psum_h_{j}")
                psum_h = ph_of[j]
                s_col0 = blk_off[j] * BLK
                for k in range(cs):
                    t = base + k
                    nc.tensor.matmul(
                        psum_h[:],
                        sall_t[:, s_col0 + t * BLK: s_col0 + (t + 1) * BLK],
                        m_sb[:, k * d_in:(k + 1) * d_in],
                        start=(t == 0), stop=(t == Tj - 1))
                if base + cs == Tj:  # block finished -> drain to SBUF strip
                    if bench_no_drain:
                        ph_of.pop(j)
                        return
                    nc.vector.tensor_copy(h_all[:, j * d_in:(j + 1) * d_in],
                                          psum_h[:])
                    ph_of.pop(j)
                    done_blocks += 1
                    if done_blocks == n_blocks:
                        nc.scalar.dma_start(h_d.ap(), h_all[:])

            if filler:
                # HAM keep-warm: dependency-free matmuls the in-order PE can
                # run during DMA-wait gaps, so the clock gate stays at 8/8.
                pdum_t = pdum.tile([BLK, d_in], f32)

                def emit_filler():
                    for _ in range(filler):
                        nc.tensor.matmul(pdum_t[:], sall_t[:, :BLK],
                                         wt_t[:, :d_in], start=True,
                                         stop=True, skip_group_check=True)

            for q in range(NCH + SKEW):
                if q < NCH:
                    emit_filt(q)
                if q >= SKEW:
                    emit_scatter(q - SKEW)
                if filler:
                    emit_filler()

    nc.compile()
    return nc


# TileContext wrapper: single place to tweak kwargs if needed.
def TileContextCompat(nc):
    return tile.TileContext(nc)


def _kernel_impl(x, edge_basis, src, dst, W, b,
                 n_nodes, d_in, d_radial, n_cores, run_fn=None):
    dst = np.asarray(dst)
    order = np.argsort(dst, kind="stable")
    dst_sorted = dst[order]
    T_list, e_start, e_end, npc, n_blocks, perm = _plan(dst_sorted, n_nodes,
                                                        n_cores)
    TT = int(T_list.sum())

    eb_bf = np.asarray(edge_basis).astype(BF16)
    srcx = np.asarray(x)[np.asarray(src)].astype(BF16)  # x gathered per edge

    has_bias = bool(np.any(np.asarray(b) != 0))

    in_maps = []
    for c in range(n_cores):
        ebs, xss, idx, pad = _prepare_core(
            eb_bf, srcx, order, e_start, e_end, T_list, perm, c, npc,
            n_blocks, d_in, d_radial)
        # rel per slot: node index within the 128-node block; pads -> 0
        # (their m is exactly 0, so the scatter target is irrelevant).
        rel_slot = np.zeros(TT * BLK, np.int64)
        valid = ~pad
        rel_slot[valid] = (dst[idx[valid]] - c * npc) % BLK
        one_hot = np.zeros((TT * BLK, BLK), np.uint8)
        one_hot[np.arange(TT * BLK), rel_slot] = 1
        sall = np.ascontiguousarray(
            one_hot.reshape(TT, BLK, BLK).transpose(1, 0, 2)
            .reshape(BLK, TT * BLK)).astype(F8)
        m = {
            "ebs": ebs,
            "xss": xss,
            "sall": sall,
            "wt": np.ascontiguousarray(np.asarray(W).T).astype(BF16),
        }
        if has_bias:
            m["bb"] = np.tile(np.asarray(b).astype(np.float32),
                              (BLK, CHUNK_MAX))
        in_maps.append(m)

    nc = build_program(TT, T_list, n_blocks, d_in, d_radial, n_cores,
                       has_bias)
    global LAST_BUILD
    LAST_BUILD = (nc, in_maps)
    if run_fn is None:
        res = run_bass_kernel_spmd(nc, in_maps, core_ids=list(range(n_cores)))
        results = res.results
    else:
        results = run_fn(nc, in_maps)

    h = np.empty((n_nodes, d_in), np.float32)
    for c in range(n_cores):
        hc = results[c]["h"].reshape(BLK, n_blocks, d_in).transpose(1, 0, 2)
        blocks = np.empty_like(hc)          # un-permute loop order -> blocks
        blocks[perm[c]] = hc
        blocks = blocks.reshape(n_blocks * BLK, d_in)
        h[c * npc:(c + 1) * npc] = blocks[:npc]
    return h


def kernel(x, edge_basis, src, dst, W, b):
    assert x.shape == (N_NODES, D_IN)
    assert edge_basis.shape == (N_EDGES, D_RADIAL)
    h = _kernel_impl(x, edge_basis, src, dst, W, b,
                     N_NODES, D_IN, D_RADIAL, N_CORES)
    return h.astype(x.dtype)


# revision 31
# speedup vs baseline: 1.0991x; 1.0064x over previous
"""Trainium2 Bass kernel for GNN message passing:

    h = segment_sum(x[src] * (edge_basis @ W.T + b), dst, num_segments=N)

Strategy (node-sharded, sort-based; no collectives — each core owns its
output rows exclusively):
  - Host: sort edges by dst; core c owns the contiguous node range
    [c*N/8, (c+1)*N/8). Within a core, nodes are grouped into blocks of
    128; each block's (contiguous, because sorted) edge list is padded to
    tiles of 128 edges. Blocks are processed in per-core descending-size
    order so the shared (SPMD-identical) tile schedule T_list[j] =
    max_over_cores(j-th largest block) stays tight (~3% padding); the
    host un-permutes the output blocks afterwards. Per core the host
    materializes one packed stream (single DMA per block, issued on
    alternating HWDGE rings):
      * st  [128, TT*192] bf16 : per block, edge_basis tiles TRANSPOSED
        ([r, 128e], consumed as matmul weights) followed by x[src]
        gathered rows ([128e, 64], edge on partition).
      * sall [128, TT*128] f8  : HOST-PREBUILT one-hot scatter tiles
        S[e, n] = (dst_e - block_base == n), fp8e4 (exact 0/1), kept
        SBUF-RESIDENT (loaded once, outside the steady-state loop —
        topology-derived constant, reusable across passes).
      * wt  [128, 64]     bf16 : W.T (matmul rhs, resident).
    Pad edges have eb = 0, xs = 0 -> contribute exactly 0.
  - Device per tile (128 edges), software-pipelined with SKEW chunks:
      PE:  filt[128e,64d](psum) = ebT_tile.T @ WT   (FWL bf16 weight swap)
      DVE: m(bf16 sbuf) = xs * filt  -- reads PSUM f32 directly,
           chunk-batched (no ACT copy stage)
      PE:  psum_h[128n,64d] += S_tile.T @ m         (fp8 FWL weight swap,
           accumulate per block)
    Per block: ACT copies psum_h into a resident SBUF strip; every 8
    blocks one DMA stores that h slice (overlapped with compute); host
    de-interleaves to [N/8, 64].
  Accuracy: inputs quantized to bf16, one-hot exact in fp8, accumulation
  in f32 PSUM; rel RMS error ~3.5e-3 vs the f32 reference.
"""

import math
from contextlib import ExitStack

import numpy as np
import ml_dtypes

import concourse.bass as bass
import concourse.bacc as bacc
import concourse.tile as tile
from concourse import mybir
from concourse.bass_utils import run_bass_kernel_spmd

BF16 = ml_dtypes.bfloat16
F8 = ml_dtypes.float8_e4m3  # mybir.dt.float8e4

# Problem configuration (hardcoded per the task spec).
N_NODES = 50000
N_EDGES = 800000
D_IN = 64
D_RADIAL = 128
N_CORES = 8

LAST_BUILD = None  # (nc, in_maps) of the most recent build, for test harnesses

BLK = 128          # nodes per block (= one-hot width = psum partition dim)
CHUNK_MAX = 8      # tiles per DVE batch (psum_filt = 1 bank)
SKEW = 2           # software-pipeline skew, in chunks
H_DMA_BLKS = 8     # store h strip every this many finished blocks
SORT_BLOCKS = True


def _plan(dst_sorted, n_nodes, n_cores):
    """Compute per-(core, block) edge ranges and the shared tile schedule.

    Returns (T_list, e_start, e_end, npc, n_blocks, perm):
      T_list[j]  = tiles allocated for local block j (same for all cores)
      e_start/e_end[c, j] = edge index range (into the sorted edge order)
    """
    npc = n_nodes // n_cores
    assert npc * n_cores == n_nodes
    n_blocks = math.ceil(npc / BLK)
    bounds = np.empty((n_cores, n_blocks + 1), np.int64)
    for c in range(n_cores):
        for j in range(n_blocks + 1):
            bounds[c, j] = c * npc + min(j * BLK, npc)
    e_bounds = np.searchsorted(dst_sorted, bounds.ravel()).reshape(bounds.shape)
    e_start = e_bounds[:, :-1]
    e_end = e_bounds[:, 1:]
    counts = e_end - e_start
    tiles_needed = np.maximum((counts + BLK - 1) // BLK, 1)
    if SORT_BLOCKS:
        # Each core processes its blocks in descending-size order; loop
        # position j is sized by the max over cores of the j-th largest
        # block, which is much tighter than max over cores per block.
        perm = np.argsort(-tiles_needed, axis=1, kind="stable")
        sorted_tiles = np.take_along_axis(tiles_needed, perm, axis=1)
        T_list = sorted_tiles.max(axis=0)
    else:
        perm = np.tile(np.arange(n_blocks), (n_cores, 1))
        T_list = tiles_needed.max(axis=0)
    return T_list, e_start, e_end, npc, n_blocks, perm


def _prepare_core(eb_bf, srcx, order, e_start, e_end, T_list, perm,
                  core, npc, n_blocks, d_in, d_radial):
    """Build the per-core device input arrays (see module docstring)."""
    TT = int(T_list.sum())
    idx = np.full(TT * BLK, -1, np.int64)  # into sorted-edge order
    off = 0
    for j in range(n_blocks):
        blk = int(perm[core, j])
        s, e = e_start[core, blk], e_end[core, blk]
        idx[off * BLK: off * BLK + (e - s)] = order[s:e]
        off += int(T_list[j])
    pad = idx < 0
    idxc = np.where(pad, 0, idx)

    # ebT tiles: [TT, 128e, d_radial] -> [d_radial, TT, 128e]
    ebg = eb_bf[idxc]
    ebg[pad] = 0
    ebT = ebg.reshape(TT, BLK, d_radial).transpose(2, 0, 1)

    # xs tiles: [TT, 128e, d_in] -> [128e, TT, d_in]
    xsg = srcx[idxc]                  # x[src] per original edge id
    xsg[pad] = 0
    xs = xsg.reshape(TT, BLK, d_in).transpose(1, 0, 2)

    ebs = np.ascontiguousarray(ebT).reshape(BLK, TT * BLK)
    xss = np.ascontiguousarray(xs).reshape(BLK, TT * d_in)
    return ebs, xss, idx, pad


def _chunks(T_list, n_blocks):
    """Flat chunk schedule: (block j, tile base within block, chunk size)."""
    out = []
    for j in range(n_blocks):
        Tj = int(T_list[j])
        n_chunks = math.ceil(Tj / CHUNK_MAX)
        chunk = math.ceil(Tj / n_chunks)
        base = 0
        while base < Tj:
            cs = min(chunk, Tj - base)
            out.append((j, base, cs, Tj))
            base += cs
    return out


def build_program(TT, T_list, n_blocks, d_in=D_IN, d_radial=D_RADIAL,
                  n_cores=N_CORES, has_bias=False, repeat=1, loop_n=0,
                  bench_no_dma=False, bench_no_compute=False,
                  bench_no_mul=False, bench_no_scatter=False,
                  bench_no_filt=False, bench_no_drain=False, filler=0):
    """Build + compile the SPMD Bass program (identical across cores)."""
    nc = bacc.Bacc("TRN2", target_bir_lowering=False, debug=False,
                   num_devices=n_cores)
    bf = mybir.dt.bfloat16
    f32 = mybir.dt.float32
    f8 = mybir.dt.float8e4

    eb_d = nc.dram_tensor("ebs", [BLK, TT * BLK], bf, kind="ExternalInput")
    xs_d = nc.dram_tensor("xss", [BLK, TT * d_in], bf, kind="ExternalInput")
    sall_d = nc.dram_tensor("sall", [BLK, TT * BLK], f8, kind="ExternalInput")
    wt_d = nc.dram_tensor("wt", [d_radial, d_in], bf, kind="ExternalInput")
    if has_bias:
        bb_d = nc.dram_tensor("bb", [BLK, CHUNK_MAX * d_in], f32,
                              kind="ExternalInput")
    h_d = nc.dram_tensor("h", [BLK, n_blocks * d_in], f32, kind="ExternalOutput")

    T_cap = int(max(T_list))
    sched = _chunks(T_list, n_blocks)
    NCH = len(sched)
    # Per-chunk tile-offset of its block start in the global tile order.
    blk_off = np.concatenate([[0], np.cumsum(T_list)]).astype(int)

    with TileContextCompat(nc) as tc, ExitStack() as ctx:
        const = ctx.enter_context(tc.tile_pool(name="const", bufs=1))
        ebp = ctx.enter_context(tc.tile_pool(name="ebp", bufs=6))
        xsp = ctx.enter_context(tc.tile_pool(name="xsp", bufs=8))
        msb = ctx.enter_context(tc.tile_pool(name="msb", bufs=SKEW + 3))
        pfil = ctx.enter_context(
            tc.tile_pool(name="pfil", bufs=SKEW + 3, space="PSUM"))
        ph = ctx.enter_context(tc.tile_pool(name="ph", bufs=3, space="PSUM"))
        if filler:
            pdum = ctx.enter_context(
                tc.tile_pool(name="pdum", bufs=1, space="PSUM"))

        # Resident constants: loaded ONCE, outside the steady-state loop.
        wt_t = const.tile([d_radial, d_in], bf)
        nc.sync.dma_start(wt_t[:], wt_d.ap())
        sall_t = const.tile([BLK, TT * BLK], f8)
        nc.scalar.dma_start(sall_t[:], sall_d.ap())
        if has_bias:
            bb_t = const.tile([BLK, CHUNK_MAX * d_in], f32)
            nc.sync.dma_start(bb_t[:], bb_d.ap())
        h_all = const.tile([BLK, n_blocks * d_in], f32)
        if bench_no_dma:
            eb_shared = const.tile([BLK, T_cap * BLK], bf)
            nc.vector.memset(eb_shared[:], 0)
            xs_shared = const.tile([BLK, T_cap * d_in], bf)
            nc.vector.memset(xs_shared[:], 0)

        import contextlib
        loop_cm = (tc.For_i(0, loop_n, 1,
                            hint_engines=(mybir.EngineType.PE,
                                          mybir.EngineType.DVE,
                                          mybir.EngineType.Activation,
                                          mybir.EngineType.SP),
                            staggered_reset=True)
                   if loop_n else contextlib.nullcontext())
        with loop_cm:
          for _rep in range(repeat):
            eb_tiles = {}    # block j -> eb tile (freed after last filt)
            xs_tiles = {}    # block j -> xs tile (freed after last mul)
            pf_of = {}       # chunk q -> psum filt tile
            ph_of = {}       # block j -> psum h tile
            done_blocks = 0  # blocks drained so far (for h out-DMA)

            def emit_filt(q):
                j, base, cs, Tj = sched[q]
                if j not in eb_tiles:
                    if bench_no_dma:
                        eb_tiles[j] = eb_shared
                        xs_tiles[j] = xs_shared
                    else:
                        eb_t = ebp.tile([BLK, T_cap * BLK], bf, tag="eb")
                        nc.sync.dma_start(
                            eb_t[:, :Tj * BLK],
                            eb_d.ap()[:, blk_off[j] * BLK:
                                      (blk_off[j] + Tj) * BLK])
                        xs_t = xsp.tile([BLK, T_cap * d_in], bf, tag="xs")
                        nc.scalar.dma_start(
                            xs_t[:, :Tj * d_in],
                            xs_d.ap()[:, blk_off[j] * d_in:
                                      (blk_off[j] + Tj) * d_in])
                        eb_tiles[j] = eb_t
                        xs_tiles[j] = xs_t
                if bench_no_compute or bench_no_filt:
                    return
                eb_t = eb_tiles[j]
                pf = pfil.tile([BLK, CHUNK_MAX * d_in], f32, tag="pf")
                for k in range(cs):
                    t = base + k
                    nc.tensor.matmul(pf[:, k * d_in:(k + 1) * d_in],
                                     eb_t[:, t * BLK:(t + 1) * BLK],
                                     wt_t[:], start=True, stop=True)
                pf_of[q] = pf

            def emit_scatter(q):
                nonlocal done_blocks
                j, base, cs, Tj = sched[q]
                if bench_no_compute or bench_no_scatter:
                    if (not bench_no_compute) and (q in pf_of):
                        pf_of.pop(q)
                    return
                xs_t = xs_tiles[j]
                pf = pf_of.pop(q) if not bench_no_filt else None
                if bench_no_mul or bench_no_filt:
                    m_sb = xs_t[:, base * d_in:(base + cs) * d_in]
                else:
                    m_sb = msb.tile([BLK, CHUNK_MAX * d_in], bf, tag="m")
                    if has_bias:
                        nc.vector.tensor_add(pf[:, :cs * d_in],
                                             pf[:, :cs * d_in],
                                             bb_t[:, :cs * d_in])
                    nc.vector.tensor_mul(
                        m_sb[:, :cs * d_in],
                        xs_t[:, base * d_in:(base + cs) * d_in],
                        pf[:, :cs * d_in])
                if j not in ph_of:
                    ph_of[j] = ph.tile([BLK, d_in], f32, tag="ph",
                                       name=f"psum_h_{j}")
                psum_h = ph_of[j]
                s_col0 = blk_off[j] * BLK
                for k in range(cs):
                    t = base + k
                    nc.tensor.matmul(
                        psum_h[:],
                        sall_t[:, s_col0 + t * BLK: s_col0 + (t + 1) * BLK],
                        m_sb[:, k * d_in:(k + 1) * d_in],
                        start=(t == 0), stop=(t == Tj - 1))
                if base + cs == Tj:  # block finished -> drain to SBUF strip
                    if bench_no_drain:
                        ph_of.pop(j)
                        return
                    nc.vector.tensor_copy(h_all[:, j * d_in:(j + 1) * d_in],
                                          psum_h[:])
                    ph_of.pop(j)
                    done_blocks += 1
                    if done_blocks == n_blocks:
                        nc.scalar.dma_start(h_d.ap(), h_all[:])

            if filler:
                # HAM keep-warm: dependency-free matmuls the in-order PE can
                # run during DMA-wait gaps, so the clock gate stays at 8/8.
                pdum_t = pdum.tile([BLK, d_in], f32)

                def emit_filler():
                    for _ in range(filler):
                        nc.tensor.matmul(pdum_t[:], sall_t[:, :BLK],
                                         wt_t[:, :d_in], start=True,
                                         stop=True, skip_group_check=True)

            for q in range(NCH + SKEW):
                if q < NCH:
                    emit_filt(q)
                if q >= SKEW:
                    emit_scatter(q - SKEW)
                if filler:
                    emit_filler()

    nc.compile()
    return nc


# TileContext wrapper: single place to tweak kwargs if needed.
def TileContextCompat(nc):
    return tile.TileContext(nc)


def _kernel_impl(x, edge_basis, src, dst, W, b,
                 n_nodes, d_in, d_radial, n_cores, run_fn=None):
    dst = np.asarray(dst)
    order = np.argsort(dst, kind="stable")
    dst_sorted = dst[order]
    T_list, e_start, e_end, npc, n_blocks, perm = _plan(dst_sorted, n_nodes,
                                                        n_cores)
    TT = int(T_list.sum())

    eb_bf = np.asarray(edge_basis).astype(BF16)
    srcx = np.asarray(x)[np.asarray(src)].astype(BF16)  # x gathered per edge

    has_bias = bool(np.any(np.asarray(b) != 0))

    in_maps = []
    for c in range(n_cores):
        ebs, xss, idx, pad = _prepare_core(
            eb_bf, srcx, order, e_start, e_end, T_list, perm, c, npc,
            n_blocks, d_in, d_radial)
        # rel per slot: node index within the 128-node block; pads -> 0
        # (their m is exactly 0, so the scatter target is irrelevant).
        rel_slot = np.zeros(TT * BLK, np.int64)
        valid = ~pad
        rel_slot[valid] = (dst[idx[valid]] - c * npc) % BLK
        one_hot = np.zeros((TT * BLK, BLK), np.uint8)
        one_hot[np.arange(TT * BLK), rel_slot] = 1
        sall = np.ascontiguousarray(
            one_hot.reshape(TT, BLK, BLK).transpose(1, 0, 2)
            .reshape(BLK, TT * BLK)).astype(F8)
        m = {
            "ebs": ebs,
            "xss": xss,
            "sall": sall,
            "wt": np.ascontiguousarray(np.asarray(W).T).astype(BF16),
        }
        if has_bias:
            m["bb"] = np.tile(np.asarray(b).astype(np.float32),
                              (BLK, CHUNK_MAX))
        in_maps.append(m)

    nc = build_program(TT, T_list, n_blocks, d_in, d_radial, n_cores,
                       has_bias)
    global LAST_BUILD
    LAST_BUILD = (nc, in_maps)
    if run_fn is None:
        res = run_bass_kernel_spmd(nc, in_maps, core_ids=list(range(n_cores)))
        results = res.results
    else:
        results = run_fn(nc, in_maps)

    h = np.empty((n_nodes, d_in), np.float32)
    for c in range(n_cores):
        hc = results[c]["h"].reshape(BLK, n_blocks, d_in).transpose(1, 0, 2)
        blocks = np.empty_like(hc)          # un-permute loop order -> blocks
        blocks[perm[c]] = hc
        blocks = blocks.reshape(n_blocks * BLK, d_in)
        h[c * npc:(c + 1) * npc] = blocks[:npc]
    return h


def kernel(x, edge_basis, src, dst, W, b):
    assert x.shape == (N_NODES, D_IN)
    assert edge_basis.shape == (N_EDGES, D_RADIAL)
    h = _kernel_impl(x, edge_basis, src, dst, W, b,
                     N_NODES, D_IN, D_RADIAL, N_CORES)
    return h.astype(x.dtype)
